# revision 9
# baseline (speedup 1.0000x reference)
"""Trainium2 Bass kernel for the Diffusion get_energy problem.

Math (per graph b, all computed on one NeuronCore; data-parallel over the
8 graphs across 8 cores):

  rot = QR(pre_rot).Q                        (host, tiny)
  new_lig[t,l] = rot[t] @ lig_coord[l] + trans[t]          (host, tiny)
  atn[l,r,e]  = sum_f lig_feat[l,e,f]*rec_feat[r,e,f] * mask[l,r]   (PE)
  d2[t,l,r]   = |new_lig[t,l] - rec_coord[r]|^2            (PE)
  U[b,t] = sum_{l,r,e} atn[l,r,e] * d(t,l,r)^exps[e],  exps=[-3,-2,-1,1,2]

d2 is emitted as ONE K=15 fp16 matmul per 512-col half: the fp32 aug
factors (new_lig|nl2|1) x (-2rec|1|rec2) are split hi/lo in fp16 and
stacked [Ah;Al;Ah] x [Bh;Bh;Bl] so the fp16 matmul reproduces fp32 d2 to
~2^-21 while running at bf16 PE rate (the old fp32 path was 2x cols).

Power strips (bf16), all three on ScalarE with the single
"abs_reciprocal_sqrt_and_small" table set (rsqrt + square), batched in
QUADS of 4 timesteps so per-op overhead amortizes:
  s1 = rsqrt(d2)      per-t from PSUM     [S]
  s2 = square(s1) = 1/d2   quad-wide      [S]
  d1 = rsqrt(s2) = sqrt(d2)  quad-wide    [S]
The d1 = rsqrt(s2) identity replaces the old b = rsqrt(s1), d1 = b*b
chain: same ScalarE pass count, but the DVE b*b pass (8.8us) vanishes.
Products: five quad-wide 2x DVE tensor_tensor ops per quad
  q0=atn3*s2, p0=q0*s1 (cube), p1=atn2*s2, p2=atn1*s1, p3=atnd*d1
with the atn channel broadcast via a stride-0 AP dim. GPSIMD is
deliberately idle: any gpsimd tensor_tensor throttles concurrent DVE
ops ~4x (measured). Reduction over l for all 4 channels: one-hot
matmuls on PE accumulating in PSUM (lig mask in the one-hot columns);
final free-axis reduce on the ScalarE accumulate port.
Channel +2 (d^2) is separable and computed analytically:
  sum a2*d2 = sum_l nl2d[t,l,:] . W[l,:],  W[l,c] = sum_f lig4[f,l] Z[f,c]
with Z[f,c] = sum_r rec4[f,r] y[r,c] folded into host prep (same O(R*F)
order as the feature-transpose prep itself).
"""

import numpy as np
import ml_dtypes

B, T, L, R, E, F = 8, 16, 128, 1024, 5, 512
KF = F // 128  # 4 f-blocks of 128
NCHIP = 8

# how many of the 4 atn PSUM->SBUF copies run on ScalarE (rest on DVE)
N_COPIES_ACT = 2
# how many of the 4 quads compute s2 = s1^2 on DVE (rest on ScalarE);
# d1 = rsqrt(s2) is ScalarE-only either way
N_S2_DVE = 1

_BUILT = None  # cached nc


# --------------------------------------------------------------------------
# device program
# --------------------------------------------------------------------------
def build_nc(repeat=1):
    from contextlib import ExitStack

    import concourse.bacc as bacc
    import concourse.mybir as mybir
    import concourse.tile as tile

    f32 = mybir.dt.float32
    bf16 = mybir.dt.bfloat16
    fp16 = mybir.dt.float16
    AF = mybir.ActivationFunctionType
    MUL = mybir.AluOpType.mult

    nc = bacc.Bacc("TRN2", target_bir_lowering=False)

    d_ligTb = nc.dram_tensor("ligTb", [128, 4 * KF * L], bf16, kind="ExternalInput")
    d_ligT4 = nc.dram_tensor("ligT4", [128, KF * L], f32, kind="ExternalInput")
    d_recTb = nc.dram_tensor("recTb", [128, 4 * KF * R], bf16, kind="ExternalInput")
    d_nlaug = nc.dram_tensor("nlaug", [15, T * L], fp16, kind="ExternalInput")
    d_recaug = nc.dram_tensor("recaug", [15, R], fp16, kind="ExternalInput")
    d_nl2d = nc.dram_tensor("nl2d", [128, 5 * T], f32, kind="ExternalInput")
    d_z = nc.dram_tensor("z", [128, KF * 5], f32, kind="ExternalInput")
    d_onehot = nc.dram_tensor("onehot", [128, T * T], bf16, kind="ExternalInput")
    d_u4 = nc.dram_tensor("u4", [16, 1], f32, kind="ExternalOutput")
    d_u2 = nc.dram_tensor("u2", [1, 16], f32, kind="ExternalOutput")

    with ExitStack() as ctx:
        tc = ctx.enter_context(tile.TileContext(nc))
        const = ctx.enter_context(tc.tile_pool(name="const", bufs=1 if repeat == 1 else 2))
        recp = ctx.enter_context(tc.tile_pool(name="recp", bufs=2))
        dcp = ctx.enter_context(tc.tile_pool(name="dcp", bufs=2))
        pcp = ctx.enter_context(tc.tile_pool(name="pcp", bufs=2))
        psA = ctx.enter_context(tc.tile_pool(name="psA", bufs=1, space="PSUM"))
        psD = ctx.enter_context(tc.tile_pool(name="psD", bufs=2, space="PSUM"))
        psU = ctx.enter_context(tc.tile_pool(name="psU", bufs=1, space="PSUM"))

        for _rep in range(repeat):
            # ---- constant loads (tiny inputs via SWDGE on gpsimd; ligT on the
            # SP ring ahead of the recT stream) ---------------------------------
            t_ligTb = const.tile([128, 4 * KF * L], bf16)
            nc.sync.dma_start(out=t_ligTb[:], in_=d_ligTb[:])
            t_ligT4 = const.tile([128, KF * L], f32, bufs=1)
            nc.sync.dma_start(out=t_ligT4[:], in_=d_ligT4[:])
            t_nlaug = const.tile([15, T * L], fp16)
            nc.scalar.dma_start(out=t_nlaug[:], in_=d_nlaug[:])
            t_recaug = const.tile([15, R], fp16)
            nc.scalar.dma_start(out=t_recaug[:], in_=d_recaug[:])
            t_nl2d = const.tile([128, 5 * T], f32, bufs=1)
            nc.sync.dma_start(out=t_nl2d[:], in_=d_nl2d[:])
            t_z = const.tile([128, KF * 5], f32, bufs=1)
            nc.sync.dma_start(out=t_z[:], in_=d_z[:])
            t_onehot = const.tile([128, T * T], bf16)
            nc.scalar.dma_start(out=t_onehot[:], in_=d_onehot[:])

            # ---- atn coefficients ---------------------------------------------
            # channels 0..3 -> bf16 cat buffer (strip order matches exps order
            # [-3,-2,-1,+1]). rec mask is pre-applied to recTb on the host, lig
            # mask rides in the one-hot reduction columns -> plain copies.
            # Both 512-col halves of each channel land in one [128,1024] PSUM
            # tile so the drain is a single full-R copy per channel.
            t_atncat = const.tile([128, 4 * R], bf16)
            for e in range(4):
                t_rec = recp.tile([128, KF * R], bf16, tag="rec")
                nc.sync.dma_start(
                    out=t_rec[:], in_=d_recTb[:, e * KF * R : (e + 1) * KF * R]
                )
                ps_a = psA.tile([128, 1024], f32, tag="atn")
                for h in range(2):
                    for k in range(KF):
                        nc.tensor.matmul(
                            ps_a[:, h * 512 : (h + 1) * 512],
                            lhsT=t_ligTb[:, (e * KF + k) * L : (e * KF + k + 1) * L],
                            rhs=t_rec[:, k * R + h * 512 : k * R + h * 512 + 512],
                            start=(k == 0),
                            stop=(k == KF - 1),
                        )
                dst = t_atncat[:, e * R : (e + 1) * R]
                if e < N_COPIES_ACT:
                    nc.scalar.copy(out=dst, in_=ps_a[:])
                else:
                    nc.vector.tensor_copy(dst, ps_a[:])

            # ---- analytic +2 channel ------------------------------------------
            # W[l,c] = sum_f lig4[f,l] Z[f,c]; Z folded on the host (rec mask in
            # Z via y, lig mask in nl2d).  U2[t] = sum_{l,c} W[l,c] nl2d[l,c,t].
            ps_w = psA.tile([128, 5], f32, tag="aux")
            for k in range(KF):
                nc.tensor.matmul(
                    ps_w[:],
                    lhsT=t_ligT4[:, k * L : (k + 1) * L],
                    rhs=t_z[:, k * 5 : (k + 1) * 5],
                    start=(k == 0),
                    stop=(k == KF - 1),
                )
            t_w = const.tile([128, 5], f32)
            nc.scalar.copy(out=t_w[:], in_=ps_w[:])
            ps_u2 = psA.tile([1, 16], f32, tag="aux")
            for c in range(5):
                nc.tensor.matmul(
                    ps_u2[:],
                    lhsT=t_w[:, c : c + 1],
                    rhs=t_nl2d[:, c * T : (c + 1) * T],
                    start=(c == 0),
                    stop=(c == 4),
                )
            t_u2 = const.tile([1, 16], f32)
            nc.scalar.copy(out=t_u2[:], in_=ps_u2[:])
            nc.sync.dma_start(out=d_u2[:], in_=t_u2[:])

            # ---- t-loop: powers + products + reduction -------------------------
            # software-pipelined emission: d2(t+1) is issued mid-t so PE never
            # stalls behind t's reduction matmuls.
            t_upsum = psU.tile([16, 512], f32)

            def emit_d2(t, ps):
                for h in range(2):
                    nc.tensor.matmul(
                        ps[:, h * 512 : (h + 1) * 512],
                        lhsT=t_nlaug[:, t * L : (t + 1) * L],
                        rhs=t_recaug[:, h * 512 : (h + 1) * 512],
                        start=True,
                        stop=True,
                    )

            # Timesteps are processed in QUADS (4 t's per op) so per-op
            # overhead — especially gpsimd's ~800ns semaphore events, which
            # paced the per-t version at 4.8us/t — amortizes 4x.
            NQ = T // 4

            def emit_quad_strips_ps(qi):
                """d2 + S strips for t = 4qi..4qi+3:
                s1 = rsqrt(d2) per t [S, frees PSUM]; then quad-wide
                s2 = s1^2 [S] and d1 = rsqrt(s2) = sqrt(d2) [S]."""
                t_s1Q = dcp.tile([128, 4 * R], bf16, tag="s1Q", bufs=3)
                for i in range(4):
                    ps_d2 = psD.tile([128, 1024], f32, tag="d2")
                    emit_d2(4 * qi + i, ps_d2)
                    nc.scalar.activation(
                        out=t_s1Q[:, i * R : (i + 1) * R],
                        in_=ps_d2[:],
                        func=AF.Abs_reciprocal_sqrt,
                    )
                t_s2Q = dcp.tile([128, 4 * R], bf16, tag="s2Q", bufs=3)
                if qi < N_S2_DVE:
                    nc.vector.tensor_tensor(
                        out=t_s2Q[:], in0=t_s1Q[:], in1=t_s1Q[:], op=MUL
                    )
                else:
                    nc.scalar.activation(out=t_s2Q[:], in_=t_s1Q[:], func=AF.Square)
                t_d1Q = dcp.tile([128, 4 * R], bf16, tag="d1Q", bufs=3)
                nc.scalar.activation(
                    out=t_d1Q[:], in_=t_s2Q[:], func=AF.Abs_reciprocal_sqrt
                )
                return [t_s1Q, t_s2Q, t_d1Q]

            def bcast(ch):
                return (
                    t_atncat[:, ch * R : (ch + 1) * R]
                    .unsqueeze(1)
                    .broadcast_to([128, 4, R])
                )

            # NOTE: gpsimd is deliberately UNUSED in the t-loop — any gpsimd
            # tensor_tensor throttles concurrent DVE ops ~4x (SBUF contention,
            # measured), costing far more V-time than it saves.
            # strips run TWO quads ahead of consumption so the V stream
            # never stalls on the scalar rsqrt/square chain
            strips_q = [emit_quad_strips_ps(0), emit_quad_strips_ps(1)]
            for qi in range(NQ):
                if qi + 2 < NQ:
                    strips_q.append(emit_quad_strips_ps(qi + 2))
                s1Q, s2Q, d1Q = strips_q.pop(0)
                t_q0 = pcp.tile([128, 4 * R], bf16, tag="q0Q", bufs=1)
                nc.vector.tensor_tensor(out=t_q0[:], in0=s2Q[:], in1=bcast(0), op=MUL)
                t_p0 = pcp.tile([128, 4 * R], bf16, tag="p0Q")
                nc.vector.tensor_tensor(out=t_p0[:], in0=t_q0[:], in1=s1Q[:], op=MUL)
                t_p1 = pcp.tile([128, 4 * R], bf16, tag="p1Q", bufs=1)
                nc.vector.tensor_tensor(out=t_p1[:], in0=s2Q[:], in1=bcast(1), op=MUL)
                t_p2 = pcp.tile([128, 4 * R], bf16, tag="p2Q")
                nc.vector.tensor_tensor(out=t_p2[:], in0=s1Q[:], in1=bcast(2), op=MUL)
                t_p3 = pcp.tile([128, 4 * R], bf16, tag="p3Q")
                nc.vector.tensor_tensor(out=t_p3[:], in0=d1Q[:], in1=bcast(3), op=MUL)
                for i in range(4):
                    t = 4 * qi + i
                    for src in (t_p0, t_p1, t_p2, t_p3):
                        for h in range(2):
                            nc.tensor.matmul(
                                t_upsum[:],
                                lhsT=t_onehot[:, t * T : (t + 1) * T],
                                rhs=src[:, i * R + h * 512 : i * R + h * 512 + 512],
                                start=(qi == 0 and i == 0 and src is t_p0 and h == 0),
                                stop=(qi == NQ - 1 and i == 3 and src is t_p3 and h == 1),
                            )
            # final free-axis reduce of psU on DVE (keeps the Act queue free
            # at the iteration boundary: Act's next-iter strip chain would
            # otherwise stall here behind PE's last one-hot matmuls)
            t_u4 = const.tile([16, 1], f32)
            nc.vector.tensor_reduce(
                out=t_u4[:],
                in_=t_upsum[:],
                axis=mybir.AxisListType.XYZW,
                op=mybir.AluOpType.add,
            )
            nc.sync.dma_start(out=d_u4[:], in_=t_u4[:])

    nc.compile()

    return nc


# --------------------------------------------------------------------------
# host-side data prep
# --------------------------------------------------------------------------
def _hi_lo_f16(a):
    hi = a.astype(ml_dtypes.float16 if False else np.float16)
    lo = (a - hi.astype(np.float32)).astype(np.float16)
    return hi, lo


def prep_core_inputs(
    b, lig_feat, rec_feat, lig_coord, rec_coord, rot, trans, lig_counts, rec_counts
):
    """Build the in_map for core b (all numpy)."""
    f32 = np.float32
    lc = np.asarray(lig_coord[b], f32)  # [L,3]
    rc = np.asarray(rec_coord[b], f32)  # [R,3]
    new_lig = (
        np.einsum("tij,lj->tli", np.asarray(rot[b], f32), lc)
        + np.asarray(trans[b], f32)[:, None, :]
    )  # [T,L,3]
    nl2 = (new_lig.astype(f32) ** 2).sum(-1).astype(f32)  # [T,L]
    rec2 = (rc**2).sum(-1).astype(f32)  # [R]

    nlaug5 = np.empty((5, T * L), f32)
    nlaug5[0:3] = new_lig.transpose(2, 0, 1).reshape(3, T * L)
    nlaug5[3] = nl2.reshape(-1)
    nlaug5[4] = 1.0

    recaug5 = np.empty((5, R), f32)
    recaug5[0:3] = -2.0 * rc.T
    recaug5[3] = 1.0
    recaug5[4] = rec2

    # fp16 hi/lo split, stacked so one K=15 fp16 matmul = fp32 d2:
    #   d2 = Ah.Bh + Al.Bh + Ah.Bl   (Al.Bl ~ 2^-22, dropped)
    ah, al = _hi_lo_f16(nlaug5)
    bh, bl = _hi_lo_f16(recaug5)
    nlaug = np.concatenate([ah, al, ah], axis=0)  # [15, T*L]
    recaug = np.concatenate([bh, bh, bl], axis=0)  # [15, R]

    ligm = (np.arange(L) < int(lig_counts[b])).astype(f32)
    recm = (np.arange(R) < int(rec_counts[b])).astype(f32)

    lt = np.asarray(lig_feat[b], f32).transpose(1, 2, 0)  # [E,F,L]
    ligT = lt.reshape(E, KF, 128, L).transpose(2, 0, 1, 3)  # [128,E,KF,L]
    ligTb = np.ascontiguousarray(ligT[:, 0:4]).reshape(128, 4 * KF * L)
    ligTb = ligTb.astype(ml_dtypes.bfloat16)
    ligT4 = np.ascontiguousarray(ligT[:, 4]).reshape(128, KF * L)
    rt = np.asarray(rec_feat[b], f32).transpose(1, 2, 0)  # [E,F,R]
    recT = rt.reshape(E, KF, 128, R).transpose(2, 0, 1, 3)  # [128,E,KF,R]
    # rec mask pre-applied to the bf16 channels (so atn needs no device mask)
    recTb = np.ascontiguousarray(recT[:, 0:4] * recm).reshape(128, 4 * KF * R)
    recTb = recTb.astype(ml_dtypes.bfloat16)

    # lig mask folded into nl2d columns (zeroes padded-l terms of U2)
    nl2d = np.empty((128, 5, T), f32)
    nl2d[:, 0:3, :] = (-2.0 * new_lig).transpose(1, 2, 0)
    nl2d[:, 3, :] = nl2.T
    nl2d[:, 4, :] = 1.0
    nl2d *= ligm[:, None, None]
    nl2d = nl2d.reshape(128, 5 * T)

    # rec mask folded into y; Z[f,c] = sum_r rec4[f,r] y[r,c] on the host
    # (same O(R*F) order as the recT transpose above)
    y = np.empty((R, 5), f32)
    y[:, 0:3] = rc
    y[:, 3] = 1.0
    y[:, 4] = rec2
    y *= recm[:, None]
    rec4 = rt[4]  # [F, R] fp32
    z = (rec4 @ y).astype(f32)  # [F, 5]
    zdev = np.ascontiguousarray(z.reshape(KF, 128, 5).transpose(1, 0, 2)).reshape(
        128, KF * 5
    )

    # lig mask folded into the one-hot reduction columns
    oh = np.zeros((128, T, T), f32)
    oh[:, np.arange(T), np.arange(T)] = ligm[:, None]
    onehot = oh.reshape(128, T * T).astype(ml_dtypes.bfloat16)

    return {
        "ligTb": ligTb,
        "ligT4": ligT4,
        "recTb": recTb,
        "nlaug": nlaug,
        "recaug": recaug,
        "nl2d": nl2d,
        "z": zdev,
        "onehot": onehot,
    }


def host_rot(pre_rot):
    return np.linalg.qr(np.asarray(pre_rot, np.float32))[0]


# --------------------------------------------------------------------------
# entry point
# --------------------------------------------------------------------------
def kernel(
    lig_feat, rec_feat, lig_coord, rec_coord, pre_rot, trans, lig_counts, rec_counts
):
    global _BUILT
    from concourse.bass_utils import run_bass_kernel_spmd

    if _BUILT is None:
        _BUILT = build_nc()
    nc = _BUILT

    rot = host_rot(pre_rot)
    in_maps = [
        prep_core_inputs(
            b,
            lig_feat,
            rec_feat,
            lig_coord,
            rec_coord,
            rot,
            trans,
            lig_counts,
            rec_counts,
        )
        for b in range(B)
    ]
    res = run_bass_kernel_spmd(nc, in_maps, core_ids=list(range(NCHIP))).results
    out = np.empty((B, T), np.float32)
    for b in range(B):
        out[b] = res[b]["u4"][:, 0] + res[b]["u2"][0, :]
    return out


# --------------------------------------------------------------------------
# pure-numpy emulation of the device algorithm (for algebra validation)
# --------------------------------------------------------------------------
def kernel_numpy_emul(
    lig_feat, rec_feat, lig_coord, rec_coord, pre_rot, trans, lig_counts, rec_counts
):
    bf = ml_dtypes.bfloat16
    rot = host_rot(pre_rot)
    out = np.empty((B, T), np.float32)
    for b in range(B):
        m = prep_core_inputs(
            b,
            lig_feat,
            rec_feat,
            lig_coord,
            rec_coord,
            rot,
            trans,
            lig_counts,
            rec_counts,
        )
        ligTb = m["ligTb"].astype(np.float32).reshape(128, 4, KF, L)
        recTb = m["recTb"].astype(np.float32).reshape(128, 4, KF, R)
        atn03 = np.einsum("fekl,fekr->elr", ligTb, recTb)
        ligm = (np.arange(L) < int(lig_counts[b])).astype(np.float32)
        atncat = atn03.astype(bf)  # bf16 strips (rec-mask in recTb)
        # analytic channel
        ligT4 = m["ligT4"].reshape(128, KF, L)
        zdev = m["z"].reshape(128, KF, 5)
        W = np.einsum("fkl,fkc->lc", ligT4, zdev)
        nl2d = m["nl2d"].reshape(128, 5, T)
        u2 = np.einsum("lc,lct->t", W, nl2d)
        # power channels
        nlaug = m["nlaug"].astype(np.float32).reshape(15, T, L)
        recaug = m["recaug"].astype(np.float32)
        u4 = np.zeros(T, np.float32)
        for t in range(T):
            d2 = np.einsum("kl,kr->lr", nlaug[:, t], recaug)  # [L,R]
            s1 = (1.0 / np.sqrt(np.abs(d2))).astype(bf)
            s2 = (s1.astype(np.float32) ** 2).astype(bf)
            d1 = (1.0 / np.sqrt(np.abs(s2.astype(np.float32)))).astype(bf)
            q0 = (atncat[0].astype(np.float32) * s2.astype(np.float32)).astype(bf)
            p0 = (q0.astype(np.float32) * s1.astype(np.float32)).astype(bf)
            p1 = (atncat[1].astype(np.float32) * s2.astype(np.float32)).astype(bf)
            p2 = (atncat[2].astype(np.float32) * s1.astype(np.float32)).astype(bf)
            p3 = (atncat[3].astype(np.float32) * d1.astype(np.float32)).astype(bf)
            p = np.stack([p0, p1, p2, p3]).astype(np.float32)
            u4[t] = (ligm[None, :, None] * p).sum()
        out[b] = u4 + u2
    return out


# revision 12
# speedup vs baseline: 1.0910x; 1.0910x over previous
"""Trainium2 Bass kernel for the Diffusion get_energy problem.

Math (per graph b, all computed on one NeuronCore; data-parallel over the
8 graphs across 8 cores):

  rot = QR(pre_rot).Q                        (host, tiny)
  new_lig[t,l] = rot[t] @ lig_coord[l] + trans[t]          (host, tiny)
  atn[l,r,e]  = sum_f lig_feat[l,e,f]*rec_feat[r,e,f] * mask[l,r]   (PE)
  d2[t,l,r]   = |new_lig[t,l] - rec_coord[r]|^2            (PE)
  U[b,t] = sum_{l,r,e} atn[l,r,e] * d(t,l,r)^exps[e],  exps=[-3,-2,-1,1,2]

d2 is emitted as ONE K=15 fp16 matmul per 512-col half: the fp32 aug
factors (new_lig|nl2|1) x (-2rec|1|rec2) are split hi/lo in fp16 and
stacked [Ah;Al;Ah] x [Bh;Bh;Bl] so the fp16 matmul reproduces fp32 d2 to
~2^-21 while running at bf16 PE rate (the old fp32 path was 2x cols).

Power strips (bf16), all three on ScalarE with the single
"abs_reciprocal_sqrt_and_small" table set (rsqrt + square), batched in
QUADS of 4 timesteps so per-op overhead amortizes:
  s1 = rsqrt(d2)      per-t from PSUM     [S]
  s2 = square(s1) = 1/d2   quad-wide      [S]
  d1 = rsqrt(s2) = sqrt(d2)  quad-wide    [S]
The d1 = rsqrt(s2) identity replaces the old b = rsqrt(s1), d1 = b*b
chain: same ScalarE pass count, but the DVE b*b pass (8.8us) vanishes.
Products: five quad-wide 2x DVE tensor_tensor ops per quad
  q0=atn3*s2, p0=q0*s1 (cube), p1=atn2*s2, p2=atn1*s1, p3=atnd*d1
with the atn channel broadcast via a stride-0 AP dim. GPSIMD is
deliberately idle: any gpsimd tensor_tensor throttles concurrent DVE
ops ~4x (measured). Reduction over l for all 4 channels: one-hot
matmuls on PE accumulating in PSUM (lig mask in the one-hot columns);
final free-axis reduce on the ScalarE accumulate port.
Channel +2 (d^2) is separable and computed analytically:
  sum a2*d2 = sum_l nl2d[t,l,:] . W[l,:],  W[l,c] = sum_f lig4[f,l] Z[f,c]
with Z[f,c] = sum_r rec4[f,r] y[r,c] folded into host prep (same O(R*F)
order as the feature-transpose prep itself).
"""

import numpy as np
import ml_dtypes

B, T, L, R, E, F = 8, 16, 128, 1024, 5, 512
KF = F // 128  # 4 f-blocks of 128
NCHIP = 8

# how many of the 4 atn PSUM->SBUF copies run on ScalarE (rest on DVE)
N_COPIES_ACT = 2
# how many of the 4 quads compute s2 = s1^2 on DVE (rest on ScalarE);
# d1 = rsqrt(s2) is ScalarE-only either way
N_S2_DVE = 1

_BUILT = None  # cached nc


# --------------------------------------------------------------------------
# device program
# --------------------------------------------------------------------------
def build_nc(repeat=1):
    from contextlib import ExitStack

    import concourse.bacc as bacc
    import concourse.mybir as mybir
    import concourse.tile as tile

    f32 = mybir.dt.float32
    bf16 = mybir.dt.bfloat16
    fp16 = mybir.dt.float16
    AF = mybir.ActivationFunctionType
    MUL = mybir.AluOpType.mult

    nc = bacc.Bacc("TRN2", target_bir_lowering=False)

    d_ligTb = nc.dram_tensor("ligTb", [128, 4 * KF * L], bf16, kind="ExternalInput")
    d_ligT4 = nc.dram_tensor("ligT4", [128, KF * L], f32, kind="ExternalInput")
    d_recTb = nc.dram_tensor("recTb", [128, 4 * KF * R], bf16, kind="ExternalInput")
    d_nlaug = nc.dram_tensor("nlaug", [15, T * L], fp16, kind="ExternalInput")
    d_recaug = nc.dram_tensor("recaug", [15, R], fp16, kind="ExternalInput")
    d_nl2d = nc.dram_tensor("nl2d", [128, 5 * T], f32, kind="ExternalInput")
    d_z = nc.dram_tensor("z", [128, KF * 5], f32, kind="ExternalInput")
    d_onehot = nc.dram_tensor("onehot", [128, T * T], bf16, kind="ExternalInput")
    d_u4 = nc.dram_tensor("u4", [16, 1], f32, kind="ExternalOutput")
    d_u2 = nc.dram_tensor("u2", [1, 16], f32, kind="ExternalOutput")

    with ExitStack() as ctx:
        tc = ctx.enter_context(tile.TileContext(nc))
        const = ctx.enter_context(tc.tile_pool(name="const", bufs=1 if repeat == 1 else 2))
        recp = ctx.enter_context(tc.tile_pool(name="recp", bufs=2))
        dcp = ctx.enter_context(tc.tile_pool(name="dcp", bufs=2))
        pcp = ctx.enter_context(tc.tile_pool(name="pcp", bufs=2))
        psA = ctx.enter_context(tc.tile_pool(name="psA", bufs=1, space="PSUM"))
        psD = ctx.enter_context(tc.tile_pool(name="psD", bufs=2, space="PSUM"))
        psU = ctx.enter_context(tc.tile_pool(name="psU", bufs=1, space="PSUM"))

        for _rep in range(repeat):
            # ---- constant loads (tiny inputs via SWDGE on gpsimd; ligT on the
            # SP ring ahead of the recT stream) ---------------------------------
            t_ligTb = const.tile([128, 4 * KF * L], bf16)
            nc.sync.dma_start(out=t_ligTb[:], in_=d_ligTb[:])
            t_ligT4 = const.tile([128, KF * L], f32, bufs=1)
            nc.sync.dma_start(out=t_ligT4[:], in_=d_ligT4[:])
            t_nlaug = const.tile([15, T * L], fp16)
            nc.scalar.dma_start(out=t_nlaug[:], in_=d_nlaug[:])
            t_recaug = const.tile([15, R], fp16)
            nc.scalar.dma_start(out=t_recaug[:], in_=d_recaug[:])
            t_nl2d = const.tile([128, 5 * T], f32, bufs=1)
            nc.sync.dma_start(out=t_nl2d[:], in_=d_nl2d[:])
            t_z = const.tile([128, KF * 5], f32, bufs=1)
            nc.sync.dma_start(out=t_z[:], in_=d_z[:])
            t_onehot = const.tile([128, T * T], bf16)
            nc.scalar.dma_start(out=t_onehot[:], in_=d_onehot[:])

            # ---- atn coefficients ---------------------------------------------
            # channels 0..3 -> bf16 cat buffer (strip order matches exps order
            # [-3,-2,-1,+1]). rec mask is pre-applied to recTb on the host, lig
            # mask rides in the one-hot reduction columns -> plain copies.
            # Both 512-col halves of each channel land in one [128,1024] PSUM
            # tile so the drain is a single full-R copy per channel.
            t_atncat = const.tile([128, 4 * R], bf16)
            for e in range(4):
                t_rec = recp.tile([128, KF * R], bf16, tag="rec")
                nc.sync.dma_start(
                    out=t_rec[:], in_=d_recTb[:, e * KF * R : (e + 1) * KF * R]
                )
                ps_a = psA.tile([128, 1024], f32, tag="atn")
                for h in range(2):
                    for k in range(KF):
                        nc.tensor.matmul(
                            ps_a[:, h * 512 : (h + 1) * 512],
                            lhsT=t_ligTb[:, (e * KF + k) * L : (e * KF + k + 1) * L],
                            rhs=t_rec[:, k * R + h * 512 : k * R + h * 512 + 512],
                            start=(k == 0),
                            stop=(k == KF - 1),
                        )
                dst = t_atncat[:, e * R : (e + 1) * R]
                if e < N_COPIES_ACT:
                    nc.scalar.copy(out=dst, in_=ps_a[:])
                else:
                    nc.vector.tensor_copy(dst, ps_a[:])

            # ---- analytic +2 channel ------------------------------------------
            # W[l,c] = sum_f lig4[f,l] Z[f,c]; Z folded on the host (rec mask in
            # Z via y, lig mask in nl2d).  U2[t] = sum_{l,c} W[l,c] nl2d[l,c,t].
            ps_w = psA.tile([128, 5], f32, tag="aux")
            for k in range(KF):
                nc.tensor.matmul(
                    ps_w[:],
                    lhsT=t_ligT4[:, k * L : (k + 1) * L],
                    rhs=t_z[:, k * 5 : (k + 1) * 5],
                    start=(k == 0),
                    stop=(k == KF - 1),
                )
            t_w = const.tile([128, 5], f32)
            nc.scalar.copy(out=t_w[:], in_=ps_w[:])
            ps_u2 = psA.tile([1, 16], f32, tag="aux")
            for c in range(5):
                nc.tensor.matmul(
                    ps_u2[:],
                    lhsT=t_w[:, c : c + 1],
                    rhs=t_nl2d[:, c * T : (c + 1) * T],
                    start=(c == 0),
                    stop=(c == 4),
                )
            t_u2 = const.tile([1, 16], f32)
            nc.scalar.copy(out=t_u2[:], in_=ps_u2[:])
            nc.sync.dma_start(out=d_u2[:], in_=t_u2[:])

            # ---- t-loop: powers + products + reduction -------------------------
            # software-pipelined emission: d2(t+1) is issued mid-t so PE never
            # stalls behind t's reduction matmuls.
            t_upsum = psU.tile([16, 512], f32)

            def emit_d2(t, ps):
                for h in range(2):
                    nc.tensor.matmul(
                        ps[:, h * 512 : (h + 1) * 512],
                        lhsT=t_nlaug[:, t * L : (t + 1) * L],
                        rhs=t_recaug[:, h * 512 : (h + 1) * 512],
                        start=True,
                        stop=True,
                    )

            # Timesteps are processed in QUADS (4 t's per op) so per-op
            # overhead — especially gpsimd's ~800ns semaphore events, which
            # paced the per-t version at 4.8us/t — amortizes 4x.
            NQ = T // 4

            def emit_quad_strips_ps(qi):
                """d2 + S strips for t = 4qi..4qi+3:
                s1 = rsqrt(d2) per t [S, frees PSUM]; then quad-wide
                s2 = s1^2 [S] and d1 = rsqrt(s2) = sqrt(d2) [S]."""
                t_s1Q = dcp.tile([128, 4 * R], bf16, tag="s1Q", bufs=3)
                for i in range(4):
                    ps_d2 = psD.tile([128, 1024], f32, tag="d2")
                    emit_d2(4 * qi + i, ps_d2)
                    nc.scalar.activation(
                        out=t_s1Q[:, i * R : (i + 1) * R],
                        in_=ps_d2[:],
                        func=AF.Abs_reciprocal_sqrt,
                    )
                t_s2Q = dcp.tile([128, 4 * R], bf16, tag="s2Q", bufs=3)
                if qi < N_S2_DVE:
                    nc.vector.tensor_tensor(
                        out=t_s2Q[:], in0=t_s1Q[:], in1=t_s1Q[:], op=MUL
                    )
                else:
                    nc.scalar.activation(out=t_s2Q[:], in_=t_s1Q[:], func=AF.Square)
                t_d1Q = dcp.tile([128, 4 * R], bf16, tag="d1Q", bufs=3)
                nc.scalar.activation(
                    out=t_d1Q[:], in_=t_s2Q[:], func=AF.Abs_reciprocal_sqrt
                )
                return [t_s1Q, t_s2Q, t_d1Q]

            def bcast(ch):
                return (
                    t_atncat[:, ch * R : (ch + 1) * R]
                    .unsqueeze(1)
                    .broadcast_to([128, 4, R])
                )

            # NOTE: gpsimd is deliberately UNUSED in the t-loop — any gpsimd
            # tensor_tensor throttles concurrent DVE ops ~4x (SBUF contention,
            # measured), costing far more V-time than it saves.
            # strips run TWO quads ahead of consumption so the V stream
            # never stalls on the scalar rsqrt/square chain
            strips_q = [emit_quad_strips_ps(0), emit_quad_strips_ps(1)]
            for qi in range(NQ):
                if qi + 2 < NQ:
                    strips_q.append(emit_quad_strips_ps(qi + 2))
                s1Q, s2Q, d1Q = strips_q.pop(0)
                t_q0 = pcp.tile([128, 4 * R], bf16, tag="q0Q", bufs=1)
                nc.vector.tensor_tensor(out=t_q0[:], in0=s2Q[:], in1=bcast(0), op=MUL)
                t_p0 = pcp.tile([128, 4 * R], bf16, tag="p0Q")
                nc.vector.tensor_tensor(out=t_p0[:], in0=t_q0[:], in1=s1Q[:], op=MUL)
                t_p1 = pcp.tile([128, 4 * R], bf16, tag="p1Q", bufs=2)
                nc.vector.tensor_tensor(out=t_p1[:], in0=s2Q[:], in1=bcast(1), op=MUL)
                t_p2 = pcp.tile([128, 4 * R], bf16, tag="p2Q")
                nc.vector.tensor_tensor(out=t_p2[:], in0=s1Q[:], in1=bcast(2), op=MUL)
                t_p3 = pcp.tile([128, 4 * R], bf16, tag="p3Q")
                nc.vector.tensor_tensor(out=t_p3[:], in0=d1Q[:], in1=bcast(3), op=MUL)
                for i in range(4):
                    t = 4 * qi + i
                    for src in (t_p0, t_p1, t_p2, t_p3):
                        for h in range(2):
                            nc.tensor.matmul(
                                t_upsum[:],
                                lhsT=t_onehot[:, t * T : (t + 1) * T],
                                rhs=src[:, i * R + h * 512 : i * R + h * 512 + 512],
                                start=(qi == 0 and i == 0 and src is t_p0 and h == 0),
                                stop=(qi == NQ - 1 and i == 3 and src is t_p3 and h == 1),
                            )
            # final free-axis reduce of psU on the Scalar accumulate port
            # (NOT DVE: DVE PSUM reads are slow on HW and stall the product
            # queue at the iteration boundary — measured +9.6us)
            t_u4dummy = const.tile([16, 512], f32, bufs=1)
            t_u4 = const.tile([16, 1], f32)
            nc.scalar.activation(
                out=t_u4dummy[:],
                in_=t_upsum[:],
                func=AF.Copy,
                accum_out=t_u4[:],
            )
            nc.sync.dma_start(out=d_u4[:], in_=t_u4[:])

    nc.compile()

    return nc


# --------------------------------------------------------------------------
# host-side data prep
# --------------------------------------------------------------------------
def _hi_lo_f16(a):
    hi = a.astype(ml_dtypes.float16 if False else np.float16)
    lo = (a - hi.astype(np.float32)).astype(np.float16)
    return hi, lo


def prep_core_inputs(
    b, lig_feat, rec_feat, lig_coord, rec_coord, rot, trans, lig_counts, rec_counts
):
    """Build the in_map for core b (all numpy)."""
    f32 = np.float32
    lc = np.asarray(lig_coord[b], f32)  # [L,3]
    rc = np.asarray(rec_coord[b], f32)  # [R,3]
    new_lig = (
        np.einsum("tij,lj->tli", np.asarray(rot[b], f32), lc)
        + np.asarray(trans[b], f32)[:, None, :]
    )  # [T,L,3]
    nl2 = (new_lig.astype(f32) ** 2).sum(-1).astype(f32)  # [T,L]
    rec2 = (rc**2).sum(-1).astype(f32)  # [R]

    nlaug5 = np.empty((5, T * L), f32)
    nlaug5[0:3] = new_lig.transpose(2, 0, 1).reshape(3, T * L)
    nlaug5[3] = nl2.reshape(-1)
    nlaug5[4] = 1.0

    recaug5 = np.empty((5, R), f32)
    recaug5[0:3] = -2.0 * rc.T
    recaug5[3] = 1.0
    recaug5[4] = rec2

    # fp16 hi/lo split, stacked so one K=15 fp16 matmul = fp32 d2:
    #   d2 = Ah.Bh + Al.Bh + Ah.Bl   (Al.Bl ~ 2^-22, dropped)
    ah, al = _hi_lo_f16(nlaug5)
    bh, bl = _hi_lo_f16(recaug5)
    nlaug = np.concatenate([ah, al, ah], axis=0)  # [15, T*L]
    recaug = np.concatenate([bh, bh, bl], axis=0)  # [15, R]

    ligm = (np.arange(L) < int(lig_counts[b])).astype(f32)
    recm = (np.arange(R) < int(rec_counts[b])).astype(f32)

    lt = np.asarray(lig_feat[b], f32).transpose(1, 2, 0)  # [E,F,L]
    ligT = lt.reshape(E, KF, 128, L).transpose(2, 0, 1, 3)  # [128,E,KF,L]
    ligTb = np.ascontiguousarray(ligT[:, 0:4]).reshape(128, 4 * KF * L)
    ligTb = ligTb.astype(ml_dtypes.bfloat16)
    ligT4 = np.ascontiguousarray(ligT[:, 4]).reshape(128, KF * L)
    rt = np.asarray(rec_feat[b], f32).transpose(1, 2, 0)  # [E,F,R]
    recT = rt.reshape(E, KF, 128, R).transpose(2, 0, 1, 3)  # [128,E,KF,R]
    # rec mask pre-applied to the bf16 channels (so atn needs no device mask)
    recTb = np.ascontiguousarray(recT[:, 0:4] * recm).reshape(128, 4 * KF * R)
    recTb = recTb.astype(ml_dtypes.bfloat16)

    # lig mask folded into nl2d columns (zeroes padded-l terms of U2)
    nl2d = np.empty((128, 5, T), f32)
    nl2d[:, 0:3, :] = (-2.0 * new_lig).transpose(1, 2, 0)
    nl2d[:, 3, :] = nl2.T
    nl2d[:, 4, :] = 1.0
    nl2d *= ligm[:, None, None]
    nl2d = nl2d.reshape(128, 5 * T)

    # rec mask folded into y; Z[f,c] = sum_r rec4[f,r] y[r,c] on the host
    # (same O(R*F) order as the recT transpose above)
    y = np.empty((R, 5), f32)
    y[:, 0:3] = rc
    y[:, 3] = 1.0
    y[:, 4] = rec2
    y *= recm[:, None]
    rec4 = rt[4]  # [F, R] fp32
    z = (rec4 @ y).astype(f32)  # [F, 5]
    zdev = np.ascontiguousarray(z.reshape(KF, 128, 5).transpose(1, 0, 2)).reshape(
        128, KF * 5
    )

    # lig mask folded into the one-hot reduction columns
    oh = np.zeros((128, T, T), f32)
    oh[:, np.arange(T), np.arange(T)] = ligm[:, None]
    onehot = oh.reshape(128, T * T).astype(ml_dtypes.bfloat16)

    return {
        "ligTb": ligTb,
        "ligT4": ligT4,
        "recTb": recTb,
        "nlaug": nlaug,
        "recaug": recaug,
        "nl2d": nl2d,
        "z": zdev,
        "onehot": onehot,
    }


def host_rot(pre_rot):
    return np.linalg.qr(np.asarray(pre_rot, np.float32))[0]


# --------------------------------------------------------------------------
# entry point
# --------------------------------------------------------------------------
def kernel(
    lig_feat, rec_feat, lig_coord, rec_coord, pre_rot, trans, lig_counts, rec_counts
):
    global _BUILT
    from concourse.bass_utils import run_bass_kernel_spmd

    if _BUILT is None:
        _BUILT = build_nc()
    nc = _BUILT

    rot = host_rot(pre_rot)
    in_maps = [
        prep_core_inputs(
            b,
            lig_feat,
            rec_feat,
            lig_coord,
            rec_coord,
            rot,
            trans,
            lig_counts,
            rec_counts,
        )
        for b in range(B)
    ]
    res = run_bass_kernel_spmd(nc, in_maps, core_ids=list(range(NCHIP))).results
    out = np.empty((B, T), np.float32)
    for b in range(B):
        out[b] = res[b]["u4"][:, 0] + res[b]["u2"][0, :]
    return out


# --------------------------------------------------------------------------
# pure-numpy emulation of the device algorithm (for algebra validation)
# --------------------------------------------------------------------------
def kernel_numpy_emul(
    lig_feat, rec_feat, lig_coord, rec_coord, pre_rot, trans, lig_counts, rec_counts
):
    bf = ml_dtypes.bfloat16
    rot = host_rot(pre_rot)
    out = np.empty((B, T), np.float32)
    for b in range(B):
        m = prep_core_inputs(
            b,
            lig_feat,
            rec_feat,
            lig_coord,
            rec_coord,
            rot,
            trans,
            lig_counts,
            rec_counts,
        )
        ligTb = m["ligTb"].astype(np.float32).reshape(128, 4, KF, L)
        recTb = m["recTb"].astype(np.float32).reshape(128, 4, KF, R)
        atn03 = np.einsum("fekl,fekr->elr", ligTb, recTb)
        ligm = (np.arange(L) < int(lig_counts[b])).astype(np.float32)
        atncat = atn03.astype(bf)  # bf16 strips (rec-mask in recTb)
        # analytic channel
        ligT4 = m["ligT4"].reshape(128, KF, L)
        zdev = m["z"].reshape(128, KF, 5)
        W = np.einsum("fkl,fkc->lc", ligT4, zdev)
        nl2d = m["nl2d"].reshape(128, 5, T)
        u2 = np.einsum("lc,lct->t", W, nl2d)
        # power channels
        nlaug = m["nlaug"].astype(np.float32).reshape(15, T, L)
        recaug = m["recaug"].astype(np.float32)
        u4 = np.zeros(T, np.float32)
        for t in range(T):
            d2 = np.einsum("kl,kr->lr", nlaug[:, t], recaug)  # [L,R]
            s1 = (1.0 / np.sqrt(np.abs(d2))).astype(bf)
            s2 = (s1.astype(np.float32) ** 2).astype(bf)
            d1 = (1.0 / np.sqrt(np.abs(s2.astype(np.float32)))).astype(bf)
            q0 = (atncat[0].astype(np.float32) * s2.astype(np.float32)).astype(bf)
            p0 = (q0.astype(np.float32) * s1.astype(np.float32)).astype(bf)
            p1 = (atncat[1].astype(np.float32) * s2.astype(np.float32)).astype(bf)
            p2 = (atncat[2].astype(np.float32) * s1.astype(np.float32)).astype(bf)
            p3 = (atncat[3].astype(np.float32) * d1.astype(np.float32)).astype(bf)
            p = np.stack([p0, p1, p2, p3]).astype(np.float32)
            u4[t] = (ligm[None, :, None] * p).sum()
        out[b] = u4 + u2
    return out


# revision 15
# speedup vs baseline: 1.5143x; 1.3880x over previous
"""Trainium2 Bass kernel for the Diffusion get_energy problem.

Math (per graph b, all computed on one NeuronCore; data-parallel over the
8 graphs across 8 cores):

  rot = QR(pre_rot).Q                        (host, tiny)
  new_lig[t,l] = rot[t] @ lig_coord[l] + trans[t]          (host, tiny)
  atn[l,r,e]  = sum_f lig_feat[l,e,f]*rec_feat[r,e,f] * mask[l,r]   (PE)
  d2[t,l,r]   = |new_lig[t,l] - rec_coord[r]|^2            (PE)
  U[b,t] = sum_{l,r,e} atn[l,r,e] * d(t,l,r)^exps[e],  exps=[-3,-2,-1,1,2]

d2 is emitted as ONE K=15 fp16 matmul per 512-col half: the fp32 aug
factors (new_lig|nl2|1) x (-2rec|1|rec2) are split hi/lo in fp16 and
stacked [Ah;Al;Ah] x [Bh;Bh;Bl] so the fp16 matmul reproduces fp32 d2 to
~2^-21 while running at bf16 PE rate (the old fp32 path was 2x cols).

Power strips (bf16), all three on ScalarE with the single
"abs_reciprocal_sqrt_and_small" table set (rsqrt + square), batched in
QUADS of 4 timesteps so per-op overhead amortizes:
  s1 = rsqrt(d2)      per-t from PSUM     [S]
  s2 = square(s1) = 1/d2   quad-wide      [S]
  d1 = rsqrt(s2) = sqrt(d2)  quad-wide    [S]
The d1 = rsqrt(s2) identity replaces the old b = rsqrt(s1), d1 = b*b
chain: same ScalarE pass count, but the DVE b*b pass (8.8us) vanishes.
Products: five quad-wide 2x DVE tensor_tensor ops per quad
  q0=atn3*s2, p0=q0*s1 (cube), p1=atn2*s2, p2=atn1*s1, p3=atnd*d1
with the atn channel broadcast via a stride-0 AP dim. GPSIMD is
deliberately idle: any gpsimd tensor_tensor throttles concurrent DVE
ops ~4x (measured). Reduction over l for all 4 channels: one-hot
matmuls on PE accumulating in PSUM (lig mask in the one-hot columns);
final free-axis reduce on the ScalarE accumulate port.
Channel +2 (d^2) is separable and computed analytically:
  sum a2*d2 = sum_l nl2d[t,l,:] . W[l,:],  W[l,c] = sum_f lig4[f,l] Z[f,c]
with Z[f,c] = sum_r rec4[f,r] y[r,c] folded into host prep (same O(R*F)
order as the feature-transpose prep itself).
"""

import numpy as np
import ml_dtypes

B, T, L, R, E, F = 8, 16, 128, 1024, 5, 512
KF = F // 128  # 4 f-blocks of 128
NCHIP = 8

# how many of the 4 atn PSUM->SBUF copies run on ScalarE (rest on DVE)
N_COPIES_ACT = 2
# how many of the 4 quads compute s2 = s1^2 on DVE (rest on ScalarE);
# d1 = rsqrt(s2) is ScalarE-only either way
N_S2_DVE = 1

_BUILT = None  # cached nc


# --------------------------------------------------------------------------
# device program
# --------------------------------------------------------------------------
def build_nc(repeat=1):
    from contextlib import ExitStack

    import concourse.bacc as bacc
    import concourse.mybir as mybir
    import concourse.tile as tile

    f32 = mybir.dt.float32
    bf16 = mybir.dt.bfloat16
    fp16 = mybir.dt.float16
    AF = mybir.ActivationFunctionType
    MUL = mybir.AluOpType.mult

    nc = bacc.Bacc("TRN2", target_bir_lowering=False)

    d_ligTb = nc.dram_tensor("ligTb", [128, 4 * KF * L], bf16, kind="ExternalInput")
    d_ligT4 = nc.dram_tensor("ligT4", [128, KF * L], f32, kind="ExternalInput")
    d_recTb = nc.dram_tensor("recTb", [128, 4 * KF * R], bf16, kind="ExternalInput")
    d_nlaug = nc.dram_tensor("nlaug", [15, T * L], fp16, kind="ExternalInput")
    d_recaug = nc.dram_tensor("recaug", [15, R], fp16, kind="ExternalInput")
    d_nl2d = nc.dram_tensor("nl2d", [128, 5 * T], f32, kind="ExternalInput")
    d_z = nc.dram_tensor("z", [128, KF * 5], f32, kind="ExternalInput")
    d_onehot = nc.dram_tensor("onehot", [128, T * T], bf16, kind="ExternalInput")
    d_u4 = nc.dram_tensor("u4", [16, 1], f32, kind="ExternalOutput")
    d_u2 = nc.dram_tensor("u2", [1, 16], f32, kind="ExternalOutput")

    with ExitStack() as ctx:
        tc = ctx.enter_context(tile.TileContext(nc))
        const = ctx.enter_context(tc.tile_pool(name="const", bufs=1 if repeat == 1 else 2))
        recp = ctx.enter_context(tc.tile_pool(name="recp", bufs=2))
        dcp = ctx.enter_context(tc.tile_pool(name="dcp", bufs=2))
        pcp = ctx.enter_context(tc.tile_pool(name="pcp", bufs=2))
        psA = ctx.enter_context(tc.tile_pool(name="psA", bufs=1, space="PSUM"))
        psD = ctx.enter_context(tc.tile_pool(name="psD", bufs=2, space="PSUM"))
        psU = ctx.enter_context(tc.tile_pool(name="psU", bufs=1, space="PSUM"))

        for _rep in range(repeat):
            # ---- constant loads (tiny inputs via SWDGE on gpsimd; ligT on the
            # SP ring ahead of the recT stream) ---------------------------------
            t_ligTb = const.tile([128, 4 * KF * L], bf16)
            nc.sync.dma_start(out=t_ligTb[:], in_=d_ligTb[:])
            t_ligT4 = const.tile([128, KF * L], f32, bufs=1)
            nc.sync.dma_start(out=t_ligT4[:], in_=d_ligT4[:])
            t_nlaug = const.tile([15, T * L], fp16)
            nc.scalar.dma_start(out=t_nlaug[:], in_=d_nlaug[:])
            t_recaug = const.tile([15, R], fp16)
            nc.scalar.dma_start(out=t_recaug[:], in_=d_recaug[:])
            t_nl2d = const.tile([128, 5 * T], f32, bufs=1)
            nc.sync.dma_start(out=t_nl2d[:], in_=d_nl2d[:])
            t_z = const.tile([128, KF * 5], f32, bufs=1)
            nc.sync.dma_start(out=t_z[:], in_=d_z[:])
            t_onehot = const.tile([128, T * T], bf16)
            nc.scalar.dma_start(out=t_onehot[:], in_=d_onehot[:])

            # ---- atn coefficients ---------------------------------------------
            # channels 0..3 -> bf16 cat buffer (strip order matches exps order
            # [-3,-2,-1,+1]). rec mask is pre-applied to recTb on the host, lig
            # mask rides in the one-hot reduction columns -> plain copies.
            # Both 512-col halves of each channel land in one [128,1024] PSUM
            # tile so the drain is a single full-R copy per channel.
            t_atncat = const.tile([128, 4 * R], bf16)
            for e in range(4):
                t_rec = recp.tile([128, KF * R], bf16, tag="rec")
                nc.sync.dma_start(
                    out=t_rec[:], in_=d_recTb[:, e * KF * R : (e + 1) * KF * R]
                )
                ps_a = psA.tile([128, 1024], f32, tag="atn")
                for h in range(2):
                    for k in range(KF):
                        nc.tensor.matmul(
                            ps_a[:, h * 512 : (h + 1) * 512],
                            lhsT=t_ligTb[:, (e * KF + k) * L : (e * KF + k + 1) * L],
                            rhs=t_rec[:, k * R + h * 512 : k * R + h * 512 + 512],
                            start=(k == 0),
                            stop=(k == KF - 1),
                        )
                dst = t_atncat[:, e * R : (e + 1) * R]
                if e < N_COPIES_ACT:
                    nc.scalar.copy(out=dst, in_=ps_a[:])
                else:
                    nc.vector.tensor_copy(dst, ps_a[:])

            # ---- analytic +2 channel ------------------------------------------
            # W[l,c] = sum_f lig4[f,l] Z[f,c]; Z folded on the host (rec mask in
            # Z via y, lig mask in nl2d).  U2[t] = sum_{l,c} W[l,c] nl2d[l,c,t].
            ps_w = psA.tile([128, 5], f32, tag="aux")
            for k in range(KF):
                nc.tensor.matmul(
                    ps_w[:],
                    lhsT=t_ligT4[:, k * L : (k + 1) * L],
                    rhs=t_z[:, k * 5 : (k + 1) * 5],
                    start=(k == 0),
                    stop=(k == KF - 1),
                )
            t_w = const.tile([128, 5], f32)
            nc.scalar.copy(out=t_w[:], in_=ps_w[:])
            ps_u2 = psA.tile([1, 16], f32, tag="aux")
            for c in range(5):
                nc.tensor.matmul(
                    ps_u2[:],
                    lhsT=t_w[:, c : c + 1],
                    rhs=t_nl2d[:, c * T : (c + 1) * T],
                    start=(c == 0),
                    stop=(c == 4),
                )
            t_u2 = const.tile([1, 16], f32)
            nc.scalar.copy(out=t_u2[:], in_=ps_u2[:])
            nc.sync.dma_start(out=d_u2[:], in_=t_u2[:])

            # ---- t-loop: powers + products + reduction -------------------------
            # software-pipelined emission: d2(t+1) is issued mid-t so PE never
            # stalls behind t's reduction matmuls.
            t_upsum = psU.tile([16, 512], f32)

            def emit_d2(t, ps):
                for h in range(2):
                    nc.tensor.matmul(
                        ps[:, h * 512 : (h + 1) * 512],
                        lhsT=t_nlaug[:, t * L : (t + 1) * L],
                        rhs=t_recaug[:, h * 512 : (h + 1) * 512],
                        start=True,
                        stop=True,
                    )

            # Timesteps are processed in QUADS (4 t's per op) so per-op
            # overhead — especially gpsimd's ~800ns semaphore events, which
            # paced the per-t version at 4.8us/t — amortizes 4x.
            NQ = T // 4

            def emit_quad_strips_ps(qi):
                """d2 + S strips for t = 4qi..4qi+3:
                s1 = rsqrt(d2) per t [S, frees PSUM]; then quad-wide
                s2 = s1^2 [S] and d1 = rsqrt(s2) = sqrt(d2) [S]."""
                t_s1Q = dcp.tile([128, 4 * R], bf16, tag="s1Q", bufs=3)
                for i in range(4):
                    ps_d2 = psD.tile([128, 1024], f32, tag="d2")
                    emit_d2(4 * qi + i, ps_d2)
                    nc.scalar.activation(
                        out=t_s1Q[:, i * R : (i + 1) * R],
                        in_=ps_d2[:],
                        func=AF.Abs_reciprocal_sqrt,
                    )
                t_s2Q = dcp.tile([128, 4 * R], bf16, tag="s2Q", bufs=3)
                if qi < N_S2_DVE:
                    nc.vector.tensor_tensor(
                        out=t_s2Q[:], in0=t_s1Q[:], in1=t_s1Q[:], op=MUL
                    )
                else:
                    nc.scalar.activation(out=t_s2Q[:], in_=t_s1Q[:], func=AF.Square)
                t_d1Q = dcp.tile([128, 4 * R], bf16, tag="d1Q", bufs=3)
                nc.scalar.activation(
                    out=t_d1Q[:], in_=t_s2Q[:], func=AF.Abs_reciprocal_sqrt
                )
                return [t_s1Q, t_s2Q, t_d1Q]

            def bcast(ch):
                return (
                    t_atncat[:, ch * R : (ch + 1) * R]
                    .unsqueeze(1)
                    .broadcast_to([128, 4, R])
                )

            # NOTE: gpsimd is deliberately UNUSED in the t-loop — any gpsimd
            # tensor_tensor throttles concurrent DVE ops ~4x (SBUF contention,
            # measured), costing far more V-time than it saves.
            # strips run TWO quads ahead of consumption so the V stream
            # never stalls on the scalar rsqrt/square chain
            strips_q = [emit_quad_strips_ps(0), emit_quad_strips_ps(1)]
            for qi in range(NQ):
                if qi + 2 < NQ:
                    strips_q.append(emit_quad_strips_ps(qi + 2))
                s1Q, s2Q, d1Q = strips_q.pop(0)
                t_q0 = pcp.tile([128, 4 * R], bf16, tag="q0Q", bufs=1)
                nc.vector.tensor_tensor(out=t_q0[:], in0=s2Q[:], in1=bcast(0), op=MUL)
                t_p0 = pcp.tile([128, 4 * R], bf16, tag="p0Q")
                nc.vector.tensor_tensor(out=t_p0[:], in0=t_q0[:], in1=s1Q[:], op=MUL)
                t_p1 = pcp.tile([128, 4 * R], bf16, tag="p1Q", bufs=1)
                nc.vector.tensor_tensor(out=t_p1[:], in0=s2Q[:], in1=bcast(1), op=MUL)
                t_p2 = pcp.tile([128, 4 * R], bf16, tag="p2Q")
                nc.vector.tensor_tensor(out=t_p2[:], in0=s1Q[:], in1=bcast(2), op=MUL)
                t_p3 = pcp.tile([128, 4 * R], bf16, tag="p3Q")
                nc.vector.tensor_tensor(out=t_p3[:], in0=d1Q[:], in1=bcast(3), op=MUL)
                for i in range(4):
                    t = 4 * qi + i
                    for src in (t_p0, t_p1, t_p2, t_p3):
                        for h in range(2):
                            nc.tensor.matmul(
                                t_upsum[:],
                                lhsT=t_onehot[:, t * T : (t + 1) * T],
                                rhs=src[:, i * R + h * 512 : i * R + h * 512 + 512],
                                start=(qi == 0 and i == 0 and src is t_p0 and h == 0),
                                stop=(qi == NQ - 1 and i == 3 and src is t_p3 and h == 1),
                            )
            # final free-axis reduce of psU on the Scalar accumulate port
            # (NOT DVE: DVE PSUM reads are slow on HW and stall the product
            # queue at the iteration boundary — measured +9.6us)
            t_u4dummy = const.tile([16, 512], f32, bufs=1)
            t_u4 = const.tile([16, 1], f32)
            nc.scalar.activation(
                out=t_u4dummy[:],
                in_=t_upsum[:],
                func=AF.Copy,
                accum_out=t_u4[:],
            )
            nc.sync.dma_start(out=d_u4[:], in_=t_u4[:])

    nc.compile()

    return nc


# --------------------------------------------------------------------------
# host-side data prep
# --------------------------------------------------------------------------
def _hi_lo_f16(a):
    hi = a.astype(ml_dtypes.float16 if False else np.float16)
    lo = (a - hi.astype(np.float32)).astype(np.float16)
    return hi, lo


def prep_core_inputs(
    b, lig_feat, rec_feat, lig_coord, rec_coord, rot, trans, lig_counts, rec_counts
):
    """Build the in_map for core b (all numpy)."""
    f32 = np.float32
    lc = np.asarray(lig_coord[b], f32)  # [L,3]
    rc = np.asarray(rec_coord[b], f32)  # [R,3]
    new_lig = (
        np.einsum("tij,lj->tli", np.asarray(rot[b], f32), lc)
        + np.asarray(trans[b], f32)[:, None, :]
    )  # [T,L,3]
    nl2 = (new_lig.astype(f32) ** 2).sum(-1).astype(f32)  # [T,L]
    rec2 = (rc**2).sum(-1).astype(f32)  # [R]

    nlaug5 = np.empty((5, T * L), f32)
    nlaug5[0:3] = new_lig.transpose(2, 0, 1).reshape(3, T * L)
    nlaug5[3] = nl2.reshape(-1)
    nlaug5[4] = 1.0

    recaug5 = np.empty((5, R), f32)
    recaug5[0:3] = -2.0 * rc.T
    recaug5[3] = 1.0
    recaug5[4] = rec2

    # fp16 hi/lo split, stacked so one K=15 fp16 matmul = fp32 d2:
    #   d2 = Ah.Bh + Al.Bh + Ah.Bl   (Al.Bl ~ 2^-22, dropped)
    ah, al = _hi_lo_f16(nlaug5)
    bh, bl = _hi_lo_f16(recaug5)
    nlaug = np.concatenate([ah, al, ah], axis=0)  # [15, T*L]
    recaug = np.concatenate([bh, bh, bl], axis=0)  # [15, R]

    ligm = (np.arange(L) < int(lig_counts[b])).astype(f32)
    recm = (np.arange(R) < int(rec_counts[b])).astype(f32)

    lt = np.asarray(lig_feat[b], f32).transpose(1, 2, 0)  # [E,F,L]
    ligT = lt.reshape(E, KF, 128, L).transpose(2, 0, 1, 3)  # [128,E,KF,L]
    ligTb = np.ascontiguousarray(ligT[:, 0:4]).reshape(128, 4 * KF * L)
    ligTb = ligTb.astype(ml_dtypes.bfloat16)
    ligT4 = np.ascontiguousarray(ligT[:, 4]).reshape(128, KF * L)
    rt = np.asarray(rec_feat[b], f32).transpose(1, 2, 0)  # [E,F,R]
    recT = rt.reshape(E, KF, 128, R).transpose(2, 0, 1, 3)  # [128,E,KF,R]
    # rec mask pre-applied to the bf16 channels (so atn needs no device mask)
    recTb = np.ascontiguousarray(recT[:, 0:4] * recm).reshape(128, 4 * KF * R)
    recTb = recTb.astype(ml_dtypes.bfloat16)

    # lig mask folded into nl2d columns (zeroes padded-l terms of U2)
    nl2d = np.empty((128, 5, T), f32)
    nl2d[:, 0:3, :] = (-2.0 * new_lig).transpose(1, 2, 0)
    nl2d[:, 3, :] = nl2.T
    nl2d[:, 4, :] = 1.0
    nl2d *= ligm[:, None, None]
    nl2d = nl2d.reshape(128, 5 * T)

    # rec mask folded into y; Z[f,c] = sum_r rec4[f,r] y[r,c] on the host
    # (same O(R*F) order as the recT transpose above)
    y = np.empty((R, 5), f32)
    y[:, 0:3] = rc
    y[:, 3] = 1.0
    y[:, 4] = rec2
    y *= recm[:, None]
    rec4 = rt[4]  # [F, R] fp32
    z = (rec4 @ y).astype(f32)  # [F, 5]
    zdev = np.ascontiguousarray(z.reshape(KF, 128, 5).transpose(1, 0, 2)).reshape(
        128, KF * 5
    )

    # lig mask folded into the one-hot reduction columns
    oh = np.zeros((128, T, T), f32)
    oh[:, np.arange(T), np.arange(T)] = ligm[:, None]
    onehot = oh.reshape(128, T * T).astype(ml_dtypes.bfloat16)

    return {
        "ligTb": ligTb,
        "ligT4": ligT4,
        "recTb": recTb,
        "nlaug": nlaug,
        "recaug": recaug,
        "nl2d": nl2d,
        "z": zdev,
        "onehot": onehot,
    }


def host_rot(pre_rot):
    return np.linalg.qr(np.asarray(pre_rot, np.float32))[0]


# --------------------------------------------------------------------------
# entry point
# --------------------------------------------------------------------------
def kernel(
    lig_feat, rec_feat, lig_coord, rec_coord, pre_rot, trans, lig_counts, rec_counts
):
    global _BUILT
    from concourse.bass_utils import run_bass_kernel_spmd

    if _BUILT is None:
        _BUILT = build_nc()
    nc = _BUILT

    rot = host_rot(pre_rot)
    in_maps = [
        prep_core_inputs(
            b,
            lig_feat,
            rec_feat,
            lig_coord,
            rec_coord,
            rot,
            trans,
            lig_counts,
            rec_counts,
        )
        for b in range(B)
    ]
    res = run_bass_kernel_spmd(nc, in_maps, core_ids=list(range(NCHIP))).results
    out = np.empty((B, T), np.float32)
    for b in range(B):
        out[b] = res[b]["u4"][:, 0] + res[b]["u2"][0, :]
    return out


# --------------------------------------------------------------------------
# pure-numpy emulation of the device algorithm (for algebra validation)
# --------------------------------------------------------------------------
def kernel_numpy_emul(
    lig_feat, rec_feat, lig_coord, rec_coord, pre_rot, trans, lig_counts, rec_counts
):
    bf = ml_dtypes.bfloat16
    rot = host_rot(pre_rot)
    out = np.empty((B, T), np.float32)
    for b in range(B):
        m = prep_core_inputs(
            b,
            lig_feat,
            rec_feat,
            lig_coord,
            rec_coord,
            rot,
            trans,
            lig_counts,
            rec_counts,
        )
        ligTb = m["ligTb"].astype(np.float32).reshape(128, 4, KF, L)
        recTb = m["recTb"].astype(np.float32).reshape(128, 4, KF, R)
        atn03 = np.einsum("fekl,fekr->elr", ligTb, recTb)
        ligm = (np.arange(L) < int(lig_counts[b])).astype(np.float32)
        atncat = atn03.astype(bf)  # bf16 strips (rec-mask in recTb)
        # analytic channel
        ligT4 = m["ligT4"].reshape(128, KF, L)
        zdev = m["z"].reshape(128, KF, 5)
        W = np.einsum("fkl,fkc->lc", ligT4, zdev)
        nl2d = m["nl2d"].reshape(128, 5, T)
        u2 = np.einsum("lc,lct->t", W, nl2d)
        # power channels
        nlaug = m["nlaug"].astype(np.float32).reshape(15, T, L)
        recaug = m["recaug"].astype(np.float32)
        u4 = np.zeros(T, np.float32)
        for t in range(T):
            d2 = np.einsum("kl,kr->lr", nlaug[:, t], recaug)  # [L,R]
            s1 = (1.0 / np.sqrt(np.abs(d2))).astype(bf)
            s2 = (s1.astype(np.float32) ** 2).astype(bf)
            d1 = (1.0 / np.sqrt(np.abs(s2.astype(np.float32)))).astype(bf)
            q0 = (atncat[0].astype(np.float32) * s2.astype(np.float32)).astype(bf)
            p0 = (q0.astype(np.float32) * s1.astype(np.float32)).astype(bf)
            p1 = (atncat[1].astype(np.float32) * s2.astype(np.float32)).astype(bf)
            p2 = (atncat[2].astype(np.float32) * s1.astype(np.float32)).astype(bf)
            p3 = (atncat[3].astype(np.float32) * d1.astype(np.float32)).astype(bf)
            p = np.stack([p0, p1, p2, p3]).astype(np.float32)
            u4[t] = (ligm[None, :, None] * p).sum()
        out[b] = u4 + u2
    return out
